# revision 14
# baseline (speedup 1.0000x reference)
"""Trainium2 Bass kernel for nn_DTAM (differential transposed-attention module).

Sharding: 8 cores = batch(4) x head(2); host merges head partial sums + residual.

Redesign vs baseline: the pointwise conv and 3x3 depthwise conv are FUSED into
9 per-tap weight matrices (host-folded), so dwq/dwk are computed directly from
a zero-padded fp8 copy of the LN-scaled input with 9 fp8 DoubleRow matmuls per
output half (contraction 192 = 2 k-tiles of 96).  The v path is fused all the
way through the attention matrix: y = sum_u ((attn * tv_u) @ Wv) @ xs_shift_u,
so V/dwv are never materialized.  All fp8 stages carry power-of-2 scales,
compensated in the softmax exp scale, the RMS epsilon scale, and the broadcast
ones-vector.

Pipeline per core:
  A: per 512-px chunk: 36 fp8-DR matmuls -> dwq/dwk psum; evac bf16 (ACT/DVE);
     per 2048-px group: DMA-xbar transpose; score matmuls accumulate in PSUM.
  B: softmax halves, attn = a1 - lam*a2, PE transpose, build 18 C_u = (attnT *
     tv_u)^T @ Wv matrices, evac to fp8.
  C: per chunk: 18 fp8-DR matmuls -> y psum; ysb fp8 evac; yy = y^2 (DVE);
     RMS stats (ones-matmul); r = rsqrt (ACT); r broadcast via K=1 matmul;
     out proj fp8-DR; osb = po * r (DVE/GPSIMD); DMA out bf16.
"""

import numpy as np
import ml_dtypes
from contextlib import ExitStack

BF16 = ml_dtypes.bfloat16
F8 = ml_dtypes.float8_e4m3

# ---- problem constants (hardcoded per contest rules) ----
B, C, H, W = 4, 192, 128, 128
HEADS = 2
N = H * W
LAM_INIT = 0.8
NCHUNK = 32          # 512-px chunks (4 image rows)
RPC = 4              # rows per chunk
GRP = 4              # chunks per transpose group
PW = W + 2           # padded width 130

# power-of-2 scales
SXS = 16.0           # xs fp8 scale
SW = 2.0 ** 16       # fused dw weight scale
SCU = 2.0 ** 18      # C_u fp8 scale
SWO = 2.0 ** 11      # Wo fp8 scale
SY = 2.0 ** -10      # ysb evac scale
SY0 = SXS * SCU      # y_psum scale
EXP_SCALE = 1.0 / (SXS * SW) ** 2
RSQ_SCALE = 1.0 / (192.0 * (SY0 * SY) ** 2)
FVAL = 1.0 / (SY0 * SY * SWO)   # folded into broadcast ones

# tap t -> (dy, dx)
OFFS = [(t // 3 - 1, t % 3 - 1) for t in range(9)]

_CACHED = {}


def _build_program():
    import concourse.bass as bass
    import concourse.bacc as bacc
    import concourse.tile as tile
    from concourse import mybir

    f32 = mybir.dt.float32
    bf16 = mybir.dt.bfloat16
    f8 = mybir.dt.float8e4
    AF = mybir.ActivationFunctionType
    OP = mybir.AluOpType
    AX = mybir.AxisListType
    DR = mybir.MatmulPerfMode.DoubleRow

    nc = bacc.Bacc("TRN2", target_bir_lowering=False, debug=False,
                   num_devices=8)

    # ---- DRAM I/O ----
    xs_d = nc.dram_tensor("xs8", [96, H, W, 2], f8, kind="ExternalInput")
    wq_d = nc.dram_tensor("wq8", [96, 9, 2, 2, 96], f8, kind="ExternalInput")
    wk_d = nc.dram_tensor("wk8", [96, 9, 2, 2, 96], f8, kind="ExternalInput")
    wv_d = nc.dram_tensor("wv_dm", [96, 2, 2, 96], bf16, kind="ExternalInput")
    dv_d = nc.dram_tensor("dv_t", [96, 2, 9], f32, kind="ExternalInput")
    wo_d = nc.dram_tensor("wo8", [96, 2, 2, 96], f8, kind="ExternalInput")
    id_d = nc.dram_tensor("ident", [96, 96], bf16, kind="ExternalInput")
    o96_d = nc.dram_tensor("ones96", [96, 1], bf16, kind="ExternalInput")
    oF_d = nc.dram_tensor("onesF", [1, 96], bf16, kind="ExternalInput")
    nl_d = nc.dram_tensor("neglam", [96, 1], f32, kind="ExternalInput")
    eps_d = nc.dram_tensor("eps", [1, 1], f32, kind="ExternalInput")
    out_d = nc.dram_tensor("out", [2, 96, N], bf16, kind="ExternalOutput")

    with tile.TileContext(nc) as tc, ExitStack() as ctx:
        cst = ctx.enter_context(tc.tile_pool(name="cst", bufs=1))
        res = ctx.enter_context(tc.tile_pool(name="res", bufs=1))

        # ---- constants ----
        wq8 = cst.tile([96, 9, 2, 2, 96], f8, name="wq8", tag="wq8")
        wk8 = cst.tile([96, 9, 2, 2, 96], f8, name="wk8", tag="wk8")
        wv_dm = cst.tile([96, 2, 2, 96], bf16, name="wvdm", tag="wvdm")
        dv_t = cst.tile([96, 2, 9], f32, name="dvt", tag="dvt")
        wo8 = cst.tile([96, 2, 2, 96], f8, name="wo8", tag="wo8")
        ident = cst.tile([96, 96], bf16, name="id", tag="id")
        ones96 = cst.tile([96, 1], bf16, name="o96", tag="o96")
        onesF = cst.tile([1, 96], bf16, name="oF", tag="oF")
        neglam = cst.tile([96, 1], f32, name="nl", tag="nl")
        eps = cst.tile([1, 1], f32, name="eps", tag="eps")
        for t_, d_ in ((wq8, wq_d), (wk8, wk_d), (wv_dm, wv_d), (dv_t, dv_d),
                       (wo8, wo_d), (ident, id_d), (ones96, o96_d),
                       (onesF, oF_d), (neglam, nl_d), (eps, eps_d)):
            nc.sync.dma_start(t_[:], d_[:])

        # ---- padded fp8 input [96, 130, 130, 2] (k-tile pairs innermost) ----
        xs = res.tile([96, H + 2, PW, 2], f8, name="xs", tag="xs")
        nc.vector.memset(xs[:, 0, :, :], 0.0)
        nc.vector.memset(xs[:, H + 1, :, :], 0.0)
        nc.vector.memset(xs[:, 1:H + 1, 0, :], 0.0)
        nc.vector.memset(xs[:, 1:H + 1, W + 1, :], 0.0)
        for i in range(8):
            r0 = 16 * i
            nc.sync.dma_start(xs[:, 1 + r0:1 + r0 + 16, 1:W + 1, :],
                              xs_d[:, r0:r0 + 16, :, :])

        # score accumulator lives in PSUM across phases A+B
        scp_ctx = ExitStack()
        scp = scp_ctx.enter_context(tc.tile_pool(name="scp", bufs=1, space="PSUM"))
        psc = scp.tile([96, 2, 512], f32, name="psc", tag="psc")

        # ================= PHASE A =================
        with tc.tile_pool(name="dwps", bufs=3, space="PSUM") as dwps, \
             tc.tile_pool(name="dws", bufs=2) as dws, \
             tc.tile_pool(name="trp", bufs=2) as trp:
            for g in range(8):
                q_sb = dws.tile([96, 2, 16, 128], bf16, name="qsb", tag="qsb")
                k_sb = dws.tile([96, 2, 16, 128], bf16, name="ksb", tag="ksb")
                for cc in range(GRP):
                    c = g * GRP + cc
                    r = RPC * c
                    tq_ps = dwps.tile([96, 2, RPC, 128], f32, name="tq",
                                      tag="dwps")
                    tk_ps = dwps.tile([96, 2, RPC, 128], f32, name="tk",
                                      tag="dwps")
                    for w8, ps in ((wq8, tq_ps), (wk8, tk_ps)):
                        for hf in range(2):
                            for t in range(9):
                                dy, dx = OFFS[t]
                                nc.tensor.matmul(
                                    ps[:, hf],
                                    w8[:, t, :, hf, :],
                                    xs[:, 1 + r + dy:1 + r + dy + RPC,
                                       1 + dx:1 + dx + W, :].rearrange(
                                           "p r x j -> p j r x"),
                                    start=(t == 0), stop=(t == 8),
                                    perf_mode=DR)
                    for hf in range(2):
                        nc.scalar.copy(q_sb[:, hf, RPC * cc:RPC * cc + RPC, :],
                                       tq_ps[:, hf])
                        nc.vector.tensor_copy(
                            k_sb[:, hf, RPC * cc:RPC * cc + RPC, :],
                            tk_ps[:, hf])
                # DMA-xbar transposes -> [128, 16, 96]
                trts = {}
                for nm, sb in (("q", q_sb), ("k", k_sb)):
                    for hf in range(2):
                        tt = trp.tile([128, 16, 96], bf16, name=f"t{nm}{hf}",
                                      tag=f"t{nm}{hf}")
                        trts[(nm, hf)] = tt
                        nc.sync.dma_start_transpose(tt[:], sb[:, hf])
                # score matmuls (PSUM-accumulated across all groups)
                for hf in range(2):
                    for blk in range(16):
                        nc.tensor.matmul(
                            psc[:, hf, 0:96],
                            trts[("q", hf)][:, blk, :],
                            trts[("k", hf)][:, blk, :],
                            start=(g == 0 and blk == 0),
                            stop=(g == 7 and blk == 15))

        # ================= PHASE B =================
        smx = ctx.enter_context(tc.tile_pool(name="smx", bufs=1))
        C8 = res.tile([96, 2, 2, 9, 96], f8, name="C8", tag="C8")
        with tc.tile_pool(name="bps", bufs=2, space="PSUM") as bps:
            nm_t = smx.tile([96, 2, 1], f32, name="nm", tag="nm")
            nms = smx.tile([96, 2, 1], f32, name="nms", tag="nms")
            e_t = smx.tile([96, 2, 96], f32, name="e", tag="e")
            sm_t = smx.tile([96, 2, 1], f32, name="sm", tag="sm")
            rr_t = smx.tile([96, 2, 1], f32, name="rr", tag="rr")
            for hf in range(2):
                nc.vector.tensor_reduce(nm_t[:, hf], psc[:, hf, 0:96], AX.X, OP.max,
                                        negate=True)
            nc.vector.tensor_scalar(nms[:], nm_t[:], EXP_SCALE, None, OP.mult)
            for hf in range(2):
                nc.scalar.activation(e_t[:, hf], psc[:, hf, 0:96], AF.Exp,
                                     bias=nms[:, hf], scale=EXP_SCALE)
                nc.vector.tensor_reduce(sm_t[:, hf], e_t[:, hf], AX.X, OP.add)
            nc.vector.reciprocal(rr_t[:], sm_t[:])
            r2n = smx.tile([96, 1], f32, name="r2n", tag="r2n")
            nc.vector.tensor_scalar(r2n[:], rr_t[:, 1], neglam[:, 0:1], None,
                                    OP.mult)
            a1 = smx.tile([96, 96], f32, name="a1", tag="a1")
            nc.scalar.mul(a1[:], e_t[:, 0], rr_t[:, 0, 0:1])
            attn = smx.tile([96, 96], bf16, name="attn", tag="attn")
            nc.vector.scalar_tensor_tensor(attn[:], e_t[:, 1], r2n[:, 0:1],
                                           a1[:], OP.mult, OP.add)
            pt = bps.tile([96, 96], bf16, name="pt", tag="pt")
            nc.tensor.transpose(pt[:], attn[:], ident[:])
            attnT = smx.tile([96, 96], bf16, name="attnT", tag="attnT")
            nc.scalar.copy(attnT[:], pt[:])
            # tmp[d, h2, u, c] = attnT[d, c] * tv[d, h2, u]
            tmp = smx.tile([96, 2, 9, 96], bf16, name="tmp", tag="tmp")
            for h2 in range(2):
                for u in range(9):
                    nc.vector.tensor_scalar(tmp[:, h2, u], attnT[:],
                                            dv_t[:, h2, u:u + 1], None,
                                            OP.mult)
            # C_u[m, c] via matmul: out[m_j, (u, c)] = sum_d Wv[d, m_j] tmp[d, u, c]
            for h2 in range(2):
                for j in range(2):
                    ca = bps.tile([96, 5, 96], f32, name="ca", tag="ca")
                    cb = bps.tile([96, 4, 96], f32, name="cb", tag="cb")
                    nc.tensor.matmul(ca[:], wv_dm[:, h2, j, :],
                                     tmp[:, h2, 0:5, :], start=True, stop=True)
                    nc.tensor.matmul(cb[:], wv_dm[:, h2, j, :],
                                     tmp[:, h2, 5:9, :], start=True, stop=True)
                    nc.scalar.mul(C8[:, h2, j, 0:5, :], ca[:], SCU)
                    nc.scalar.mul(C8[:, h2, j, 5:9, :], cb[:], SCU)
        scp_ctx.close()

        # ================= PHASE C =================
        with tc.tile_pool(name="yps", bufs=2, space="PSUM") as yps, \
             tc.tile_pool(name="sps", bufs=1, space="PSUM") as sps, \
             tc.tile_pool(name="rps", bufs=1, space="PSUM") as rps, \
             tc.tile_pool(name="pps", bufs=1, space="PSUM") as pps, \
             tc.tile_pool(name="ysp", bufs=2) as ysp, \
             tc.tile_pool(name="osp", bufs=2) as osp:
            for c in range(NCHUNK):
                r = RPC * c
                off = 512 * c
                y_ps = yps.tile([96, 2, RPC, 128], f32, name="yps", tag="yps")
                for h2 in range(2):
                    for u in range(9):
                        dy, dx = OFFS[u]
                        nc.tensor.matmul(
                            y_ps[:, h2],
                            C8[:, h2, :, u, :],
                            xs[:, 1 + r + dy:1 + r + dy + RPC,
                               1 + dx:1 + dx + W, :].rearrange(
                                   "p r x j -> p j r x"),
                            start=(u == 0), stop=(u == 8), perf_mode=DR)
                ysb = ysp.tile([96, 2, RPC, 128], f8, name="ysb", tag="ysb")
                for h2 in range(2):
                    nc.scalar.mul(ysb[:, h2], y_ps[:, h2], SY)
                yy = ysp.tile([96, 2, RPC, 128], bf16, name="yy", tag="yy")
                nc.vector.tensor_tensor(yy[:], ysb[:], ysb[:], OP.mult)
                pss = sps.tile([1, RPC, 128], f32, name="pss", tag="pss")
                nc.tensor.matmul(pss[:], ones96[:], yy[:, 0], start=True,
                                 stop=False)
                nc.tensor.matmul(pss[:], ones96[:], yy[:, 1], start=False,
                                 stop=True)
                rsb = osp.tile([1, RPC, 128], bf16, name="rsb", tag="rsb")
                nc.scalar.activation(rsb[:], pss[:], AF.Abs_reciprocal_sqrt,
                                     bias=eps[0:1, 0:1], scale=RSQ_SCALE)
                rbsb = rps.tile([96, RPC, 128], f32, name="rb", tag="rb")
                nc.tensor.matmul(rbsb[:], onesF[:], rsb[:], start=True,
                                 stop=True)
                rb_sb = osp.tile([96, RPC, 128], bf16, name="rbs", tag="rbs")
                nc.scalar.copy(rb_sb[:], rbsb[:])
                po = pps.tile([96, 2, RPC, 128], f32, name="po", tag="po")
                for oc in range(2):
                    nc.tensor.matmul(po[:, oc], wo8[:, :, oc, :], ysb[:],
                                     start=True, stop=True, perf_mode=DR)
                osb = osp.tile([96, 2, RPC, 128], bf16, name="osb", tag="osb")
                nc.vector.tensor_tensor(osb[:, 0], po[:, 0], rb_sb[:], OP.mult)
                nc.vector.tensor_tensor(osb[:, 1], po[:, 1], rb_sb[:], OP.mult)
                for oc in range(2):
                    nc.sync.dma_start(out_d[oc, :, off:off + 512], osb[:, oc])
    nc.compile()
    return nc


def _prep_inputs(inputs):
    x = np.asarray(inputs["x"], np.float32)
    norm_w = np.asarray(inputs["norm_w"], np.float32)
    Wq = np.asarray(inputs["Wq"], np.float32)
    Wk = np.asarray(inputs["Wk"], np.float32)
    Wv = np.asarray(inputs["Wv"], np.float32)
    Dq = np.asarray(inputs["Dq"], np.float32)
    Dk = np.asarray(inputs["Dk"], np.float32)
    Dv = np.asarray(inputs["Dv"], np.float32)
    t1 = np.asarray(inputs["t1"], np.float32)
    t2 = np.asarray(inputs["t2"], np.float32)
    hn_w = np.asarray(inputs["hn_w"], np.float32)
    Wo = np.asarray(inputs["Wo"], np.float32)
    lam = float(np.exp(np.sum(inputs["lq1"] * inputs["lk1"], dtype=np.float64))
                - np.exp(np.sum(inputs["lq2"] * inputs["lk2"], dtype=np.float64))
                + LAM_INIT)

    var = x.var(axis=1)
    s = 1.0 / np.sqrt(var + 1e-5)
    xs = (x * s[:, None, :, :]) * SXS          # [B, 192, H, W]
    # [B, 96, 2, H, W] fp8  (partition c96, k-tile j)
    xs8 = np.ascontiguousarray(
        xs.reshape(B, 2, 96, H, W).transpose(0, 2, 3, 4, 1)).astype(F8)

    Wq_f = Wq * norm_w[None, :]
    Wk_f = Wk * norm_w[None, :]
    Wv_f = Wv * norm_w[None, :]

    in_maps = []
    for core in range(8):
        b, h = core // 2, core % 2
        sl = slice(h * 192, (h + 1) * 192)
        m = {}
        m["xs8"] = xs8[b]
        dq = Dq[sl, 0].reshape(192, 9).copy()
        dk = Dk[sl, 0].reshape(192, 9).copy()
        dvv = Dv[sl, 0].reshape(192, 9)
        tq_s = np.sqrt(np.float32(t1[h, 0, 0]))
        tk_s = np.sqrt(np.float32(t2[h, 0, 0]))
        dq[:96] *= tq_s
        dq[96:] *= tk_s
        dk[:96] *= tq_s
        dk[96:] *= tk_s
        # w8[c96, t, j, hf, o96] = W_f[o, c] * taps[o, t] * SW, c = 96j + c96
        for nm_, Wf, taps in (("wq8", Wq_f[sl], dq), ("wk8", Wk_f[sl], dk)):
            wt = (Wf[None, :, :] * taps.T[:, :, None] * SW)   # [9t, 192o, 192c]
            wt = wt.reshape(9, 2, 96, 2, 96)                  # t, hf, o96, j, c96
            m[nm_] = np.ascontiguousarray(
                wt.transpose(4, 0, 3, 1, 2)).astype(F8)       # c96,t,j,hf,o96
        # wv_dm[d, h2, j, m96] = Wv_f[sl][96*h2 + d, 96*j + m96]
        m["wv_dm"] = np.ascontiguousarray(
            Wv_f[sl].reshape(2, 96, 2, 96).transpose(1, 0, 2, 3)).astype(BF16)
        # dv_t[d, h2, u]
        m["dv_t"] = np.ascontiguousarray(
            dvv.reshape(2, 96, 9).transpose(1, 0, 2)).astype(np.float32)
        # wo8[y96, j, oc, o96] = Wo_h[96*oc + o96, 96*j + y96] * SWO
        Wo_h = Wo[:, sl] * (hn_w[h] * (1.0 - LAM_INIT))[None, :]  # [192o,192y]
        m["wo8"] = np.ascontiguousarray(
            (Wo_h * SWO).reshape(2, 96, 2, 96).transpose(3, 2, 0, 1)).astype(F8)
        m["ident"] = np.eye(96, dtype=BF16)
        m["ones96"] = np.ones((96, 1), BF16)
        m["onesF"] = np.full((1, 96), FVAL, BF16)
        m["neglam"] = np.full((96, 1), -lam, np.float32)
        m["eps"] = np.full((1, 1), 1e-6, np.float32)
        in_maps.append(m)
    return in_maps


def kernel(**inputs):
    from concourse import bass_utils

    if "nc" not in _CACHED:
        _CACHED["nc"] = _build_program()
    nc = _CACHED["nc"]

    in_maps = _prep_inputs(inputs)
    results = bass_utils.run_bass_kernel_spmd(
        nc, in_maps, core_ids=list(range(8))).results

    x = np.asarray(inputs["x"], np.float32)
    out = np.empty((B, C, N), np.float32)
    for b in range(B):
        o0 = results[2 * b]["out"].astype(np.float32).reshape(C, N)
        o1 = results[2 * b + 1]["out"].astype(np.float32).reshape(C, N)
        out[b] = o0 + o1
    out = out.reshape(B, C, H, W) + x
    return out.astype(np.float32)
